# revision 18
# baseline (speedup 1.0000x reference)
"""Trainium2 Bass kernel: causal self-attention with ALiBi.

Problem: B=4, S=2048, E=1024, H=16, D=64 (fp32 in/out).

Sharding (8 cores): core c -> (batch b = c//2, head-group g = c%2), 8 heads
per group. Each core computes QKV projection for its batch restricted to its
heads, attention for those (b, h), and a partial output projection over its
heads' dims. Host sums the two partials per batch and adds b_out.

On-chip layout (all matmuls bf16 inputs, fp32 PSUM accumulate):
 - QKV projections produce Q and K as PAIR tiles [128, PAIRS, S]: a head
   pair's 2x64 dims live in the 128 partitions (Q pre-scaled by 1/sqrt(D)).
 - Scores for the pair are two ROW-TILED matmuls (K=64 contraction each):
   head0 uses array rows 0-63 (tile_position (0,0)), head1 rows 64-127
   ((64,0)) -- they execute concurrently in the PE array, so a chunk's two
   score matmuls cost ~one matmul of streaming time.
 - P = exp(scores) * EM on scalar/vector; EM is a host-precomputed
   [128, NH, EMW] table carrying exp(slope*(k-q)) * causal_mask.
 - V stored [128, 16, 8, 65] with a ones column so the AV matmul also
   accumulates the softmax denominator l = sum_k P as output row 64.
 - Normalization: l rows for both heads gathered into one [2, ST] tile,
   one reciprocal_approx_fast, broadcast across partitions with a single
   SBUF->SBUF DMA (0-stride source), then attn = num * (1/l) on DVE
   reading the AV psum directly.
 - Output projection psum -> SBUF copies on DVE (scalar engine is the
   attention-phase bottleneck: exp).

Baseline measured 293us; this version targets the scalar bottleneck
(exp-only) and PE score streaming (row tiling).
"""

import sys

if '/opt/trn_rl_repo' not in sys.path:
    sys.path.insert(0, '/opt/trn_rl_repo')

import numpy as np
import ml_dtypes

import concourse.bass as bass
import concourse.tile as tile
from concourse import bacc, mybir
from concourse import bass_utils

BF16 = mybir.dt.bfloat16
F32 = mybir.dt.float32
AF = mybir.ActivationFunctionType

B, S, E, H, D = 4, 2048, 1024, 16, 64
NH = 8          # heads per core
N_CORES = 8
PAIRS = NH // 2
P = 128
ST = 512        # s/q tile (free dim)
NST = S // ST   # 4
NSC = S // P    # 16 s-chunks
NCC = E // P    # 8 contraction chunks
EMW = 2048      # EM table width (q-clipped tiles)
EMOFF = 0

# ALiBi window: keys further than MARGIN/slope below the diagonal contribute
# exp(alibi) < e^-MARGIN of the max element and are skipped at chunk
# granularity.  Head h's slope is 2^-((h+1)/2).  Heads are assigned to the two
# core groups by descending window so both groups' slot s have similar cost;
# the per-slot chunk range is the UNION of the two groups' windows (the
# program is shared by all cores; EM zeros make padded chunks no-ops).
MARGIN = 12.0

SLOT_HEADS_A = [15, 13, 11, 9, 7, 5, 3, 1]
SLOT_HEADS_B = [14, 12, 10, 8, 6, 4, 2, 0]


def _jlo(h, i):
    import math as _m
    delta = MARGIN / (2.0 ** (-(h + 1) / 2.0))
    return max(0, _m.ceil((ST * i - delta - (P - 1)) / P))


def _pair_delta(pr):
    import math as _m
    return max(MARGIN / (2.0 ** (-(h + 1) / 2.0))
               for s in (2 * pr, 2 * pr + 1)
               for h in (SLOT_HEADS_A[s], SLOT_HEADS_B[s]))


def k_chunks_for_pair(pr, i):
    lo = min(_jlo(h, i) for s in (2 * pr, 2 * pr + 1)
             for h in (SLOT_HEADS_A[s], SLOT_HEADS_B[s]))
    return list(range(lo, 4 * i + 4))


def _slopes():
    x = (2.0 ** 8) ** (1.0 / H)
    return np.array([1.0 / x ** (i + 1) for i in range(H)], dtype=np.float64)


def _build_em(heads):
    """EM[p, s, c] = exp(slope_h * m) * [m <= 0], m = k - c  (h = heads[s])."""
    slopes = _slopes()
    k = np.arange(P, dtype=np.float64)[:, None]
    c = np.arange(EMW, dtype=np.float64)[None, :]
    m = k - c
    em = np.zeros((P, NH, EMW), dtype=np.float32)
    for s, h in enumerate(heads):
        v = np.exp(slopes[h] * m) * (m <= 0)
        em[:, s, :] = v.astype(np.float32)
    return em.astype(ml_dtypes.bfloat16)


_PROG_CACHE = {}


def _build_program():
    if 'prog' in _PROG_CACHE:
        return _PROG_CACHE['prog']

    nc = bacc.Bacc("TRN2", target_bir_lowering=False, debug=False,
                   num_devices=N_CORES)

    xT = nc.dram_tensor("xT", [NST, P, NCC, ST], BF16, kind="ExternalInput")
    wqk = nc.dram_tensor("wqk", [P, 2 * PAIRS, NCC, P], BF16, kind="ExternalInput")
    wv = nc.dram_tensor("wv", [P, NCC, NH * D], BF16, kind="ExternalInput")
    wout = nc.dram_tensor("wout", [P, PAIRS, E], BF16, kind="ExternalInput")
    bqk = nc.dram_tensor("bqk", [P, NCC], F32, kind="ExternalInput")
    bv = nc.dram_tensor("bv", [NH * D], BF16, kind="ExternalInput")
    em = nc.dram_tensor("em", [P, NH, EMW], BF16, kind="ExternalInput")
    out = nc.dram_tensor("out", [S, E], F32, kind="ExternalOutput")

    import math as _m

    with tile.TileContext(nc) as tc:
        with (
            tc.tile_pool(name="persist", bufs=1) as pp,
            tc.tile_pool(name="xt", bufs=3) as xtp,
            tc.tile_pool(name="pm", bufs=3) as pmp,
            tc.tile_pool(name="misc", bufs=2) as misc,
            tc.tile_pool(name="osb", bufs=2) as osb,
            tc.tile_pool(name="drp", bufs=8, space="DRAM") as drp,
        ):
            # --- weight / table DMAs, ordered so phase-1 can start ASAP ---
            # wqk is m-chunk-major on the host so the first QKV matmul only
            # waits for a 0.5MB DMA; em must land before attention round 0.
            wqk_sb = pp.tile([P, 2 * PAIRS, NCC, P], BF16, tag="wqk")
            bqk_sb = pp.tile([P, NCC], F32, tag="bqk")
            nc.sync.dma_start(wqk_sb[:, 0:1], wqk.ap()[:, 0:1])
            nc.gpsimd.dma_start(wqk_sb[:, 1:2], wqk.ap()[:, 1:2])
            nc.sync.dma_start(bqk_sb, bqk.ap())
            bv_bc = pp.tile([P, NH * D], BF16, tag="bvbc")
            nc.sync.dma_start(bv_bc, bv.ap()[None, :].to_broadcast([P, NH * D]))
            nc.sync.dma_start(wqk_sb[:, 2:4], wqk.ap()[:, 2:4])
            nc.gpsimd.dma_start(wqk_sb[:, 4:6], wqk.ap()[:, 4:6])
            nc.sync.dma_start(wqk_sb[:, 6:8], wqk.ap()[:, 6:8])
            wv_sb = pp.tile([P, NCC, NH * D], BF16, tag="wv")
            nc.gpsimd.dma_start(wv_sb, wv.ap())
            em_sb = pp.tile([P, NH, EMW], BF16, tag="em")
            for _q in range(4):
                _sl = slice(_q * 2, _q * 2 + 2)
                _eng = nc.sync if _q % 2 == 0 else nc.gpsimd
                _eng.dma_start(em_sb[:, _sl], em.ap()[:, _sl])
            wout_sb = pp.tile([P, PAIRS, E], BF16, tag="wout")
            nc.gpsimd.dma_start(wout_sb, wout.ap())

            q_sb = pp.tile([P, PAIRS, S], BF16, tag="q")
            k_sb = pp.tile([P, PAIRS, S], BF16, tag="k")
            v_sb = pp.tile([P, NSC, NH, D + 1], BF16, tag="v")
            attn_sb = pp.tile([P, PAIRS, S], BF16, tag="attn")

            nc.vector.memset(v_sb[:, :, :, D:D + 1], 1.0)

            p1 = tc.tile_pool(name="qkvps", bufs=2, space="PSUM")
            qkps = p1.__enter__()
            p2a = tc.tile_pool(name="scps", bufs=2, space="PSUM")
            scps = p2a.__enter__()
            p2b = tc.tile_pool(name="avps", bufs=1, space="PSUM")
            avps = p2b.__enter__()

            xt_tiles = {}

            def issue_xt_dma(i):
                xt = xtp.tile([P, NCC, ST], BF16, tag="xt")
                nc.scalar.dma_start(xt[:, 0:NCC // 2], xT.ap()[i][:, 0:NCC // 2])
                nc.scalar.dma_start(xt[:, NCC // 2:], xT.ap()[i][:, NCC // 2:])
                xt_tiles[i] = xt

            def qkv_steps(i):
                """Per-step closures for tile i's QKV projections."""
                xt = xt_tiles.pop(i)
                sl = slice(i * ST, (i + 1) * ST)
                st = {}
                steps = []
                for m in range(2 * PAIRS):
                    for c in range(NCC):
                        def f(m=m, c=c):
                            if c == 0:
                                st[m] = qkps.tile([P, ST], F32, tag="ps", name="ps")
                            nc.tensor.matmul(
                                st[m], lhsT=wqk_sb[:, m, c, :],
                                rhs=xt[:, c, :],
                                start=(c == 0), stop=(c == NCC - 1))
                        steps.append(f)

                    def fa(m=m):
                        pr = m // 2
                        dst = q_sb if m % 2 == 0 else k_sb
                        nc.scalar.activation(dst[:, pr, sl], st.pop(m),
                                             AF.Identity, bias=bqk_sb[:, m:m + 1])
                    steps.append(fa)
                for s4 in range(4):
                    sc = 4 * i + s4
                    for c in range(NCC):
                        def f(s4=s4, c=c):
                            if c == 0:
                                st['v'] = qkps.tile([P, NH * D], F32, tag="ps", name="ps")
                            nc.tensor.matmul(
                                st['v'], lhsT=xt[:, c, s4 * P:(s4 + 1) * P],
                                rhs=wv_sb[:, c, :],
                                start=(c == 0), stop=(c == NCC - 1))
                        steps.append(f)

                    def fv(sc=sc):
                        vp = st.pop('v')
                        nc.vector.tensor_add(
                            out=v_sb[:, sc, :, 0:D],
                            in0=vp.rearrange("p (h d) -> p h d", h=NH),
                            in1=bv_bc.rearrange("p (h d) -> p h d", h=NH))
                    steps.append(fv)
                return steps

            def outproj_steps(i):
                """Per-step closures for tile i's output projection."""
                st = {}
                steps = []
                for qc in range(4 * i, 4 * i + 4):
                    for n in range(E // ST):
                        for p_ in range(PAIRS):
                            def f(qc=qc, n=n, p_=p_):
                                if p_ == 0:
                                    st['op'] = qkps.tile([P, ST], F32, tag="ps", name="ps")
                                nc.tensor.matmul(
                                    st['op'],
                                    lhsT=attn_sb[:, p_, qc * P:(qc + 1) * P],
                                    rhs=wout_sb[:, p_, n * ST:(n + 1) * ST],
                                    start=(p_ == 0), stop=(p_ == PAIRS - 1))
                            steps.append(f)

                        def fo(qc=qc, n=n):
                            op = st.pop('op')
                            ot = osb.tile([P, ST], F32, tag="ot")
                            nc.vector.tensor_copy(ot, op)
                            nc.sync.dma_start(
                                out.ap()[qc * P:(qc + 1) * P,
                                         n * ST:(n + 1) * ST], ot)
                        steps.append(fo)
                return steps

            def attn_round(i, filler):
                """Attention for tile i, draining `filler` steps between
                matmuls to keep the PE busy through the exp/EM dep chain."""
                qsl = slice(i * ST, (i + 1) * ST)
                nslots = 2 * sum(len(k_chunks_for_pair(pr, i))
                                 for pr in range(PAIRS)) + 2 * PAIRS
                slots_left = [nslots]

                def fill():
                    k = -(-len(filler) // slots_left[0]) if filler else 0
                    for _ in range(k):
                        filler.pop(0)()
                    slots_left[0] -= 1

                for pr in range(PAIRS):
                    s0 = 2 * pr
                    delta = int(_m.ceil(_pair_delta(pr)))
                    av0 = avps.tile([P, ST], F32, tag="av0")
                    av1 = avps.tile([P, ST], F32, tag="av1")
                    js = k_chunks_for_pair(pr, i)
                    for jj, j in enumerate(js):
                        # causal clip below the diagonal; ALiBi-window clip
                        # above it (first chunk stays full-width so the AV
                        # accumulation start covers every column).
                        qlo = max(0, P * j - ST * i)
                        qhi = ST if jj == 0 else min(
                            ST, P * j + P + delta - ST * i)
                        w = qhi - qlo
                        qs = slice(i * ST + qlo, i * ST + qhi)
                        jsl = slice(j * P, (j + 1) * P)
                        scp = scps.tile([P, 2 * ST], F32, tag="scp")
                        scp3 = scp.rearrange("p (t st) -> p t st", t=2)
                        # row-tiled pair: head0 in array rows 0-63, head1 in
                        # rows 64-127 -- concurrent in the PE array.
                        nc.tensor.matmul(
                            scp[:, 0:w], lhsT=k_sb[0:D, pr, jsl],
                            rhs=q_sb[0:D, pr, qs], start=True, stop=True)
                        nc.tensor.matmul(
                            scp[:, ST:ST + w], lhsT=k_sb[D:P, pr, jsl],
                            rhs=q_sb[D:P, pr, qs], start=True, stop=True)
                        fill()
                        pm = pmp.tile([P, 2 * ST], BF16, tag="pm")
                        pm3 = pm.rearrange("p (t st) -> p t st", t=2)
                        nc.scalar.activation(pm3[:, :, 0:w], scp3[:, :, 0:w],
                                             AF.Exp)
                        pm2 = pmp.tile([P, 2 * ST], BF16, tag="pm2")
                        pm23 = pm2.rearrange("p (t st) -> p t st", t=2)
                        a0 = EMOFF - P * j + ST * i + qlo
                        nc.vector.tensor_mul(
                            pm23[:, :, 0:w], pm3[:, :, 0:w],
                            em_sb[:, s0:s0 + 2, a0:a0 + w])
                        nc.tensor.matmul(
                            av0[0:D + 1, qlo:qhi], lhsT=v_sb[:, j, s0, :],
                            rhs=pm2[:, 0:w],
                            start=(jj == 0), stop=(jj == len(js) - 1))
                        nc.tensor.matmul(
                            av1[0:D + 1, qlo:qhi], lhsT=v_sb[:, j, s0 + 1, :],
                            rhs=pm2[:, ST:ST + w],
                            start=(jj == 0), stop=(jj == len(js) - 1))
                        fill()
                    # evacuate AV psum (incl. the l row) promptly so the
                    # next pair can reuse the banks, then normalize from the
                    # SBUF copy: broadcast 1/l via DRAM round trip.
                    for odd, av in ((0, av0), (1, av1)):
                        avs = misc.tile([D + 1, ST], F32, tag=f"avs{odd}")
                        nc.vector.tensor_copy(avs, av[0:D + 1, :])
                        scratch = drp.tile([1, ST], F32, tag=f"lscr{odd}")
                        nc.sync.dma_start(scratch, avs[D:D + 1, :])
                        l_bc = misc.tile([D, ST], F32, tag=f"lbc{odd}")
                        nc.sync.dma_start(
                            l_bc,
                            bass.AP(tensor=scratch.tensor, offset=scratch.offset,
                                    ap=[[0, D]] + scratch.ap[1:]))
                        nc.vector.reciprocal_approx_fast(out=l_bc, in_=l_bc)
                        if not odd:
                            nc.vector.tensor_mul(attn_sb[0:D, pr, qsl],
                                                 avs[0:D, :], l_bc)
                        else:
                            atmp = misc.tile([D, ST], BF16, tag="atmp")
                            nc.vector.tensor_mul(atmp, avs[0:D, :], l_bc)
                            nc.sync.dma_start(attn_sb[D:P, pr, qsl], atmp)
                        fill()
                while filler:
                    filler.pop(0)()

            # software-pipelined schedule:
            #   r0: qkv(0);  r1: attn(0)+qkv(1);  r2: attn(1)+qkv(2)+outproj(0)
            #   r3: attn(2)+qkv(3)+outproj(1);  r4: attn(3)+outproj(2);
            #   tail: outproj(3)
            issue_xt_dma(0)
            issue_xt_dma(1)
            for f in qkv_steps(0):
                f()
            issue_xt_dma(2)
            attn_round(0, qkv_steps(1))
            issue_xt_dma(3)
            attn_round(1, qkv_steps(2) + outproj_steps(0))
            attn_round(2, qkv_steps(3))
            attn_round(3, outproj_steps(1) + outproj_steps(2))
            for f in outproj_steps(3):
                f()
            p2b.__exit__(None, None, None)
            p2a.__exit__(None, None, None)
            p1.__exit__(None, None, None)

    nc.compile()
    _PROG_CACHE['prog'] = nc
    return nc


def _head_groups():
    return [SLOT_HEADS_A, SLOT_HEADS_B]


def _prep_core_inputs(x, W_qkv, b_qkv, W_out, b_out):
    """Build the 8 per-core input dicts (host-side shard + transform)."""
    groups = _head_groups()
    bf = ml_dtypes.bfloat16
    per_group = []
    for heads in groups:
        qcols = []
        kcols = []
        bqk_l = []
        for p in range(PAIRS):
            h0, h1 = heads[2 * p], heads[2 * p + 1]
            wq = np.concatenate([W_qkv[:, h0 * D:(h0 + 1) * D],
                                 W_qkv[:, h1 * D:(h1 + 1) * D]], axis=1) / 8.0
            wk = np.concatenate([W_qkv[:, E + h0 * D:E + (h0 + 1) * D],
                                 W_qkv[:, E + h1 * D:E + (h1 + 1) * D]], axis=1)
            qcols.append(wq)
            kcols.append(wk)
            bqk_l.append(np.concatenate([b_qkv[h0 * D:(h0 + 1) * D],
                                         b_qkv[h1 * D:(h1 + 1) * D]]) / 8.0)
            bqk_l.append(np.concatenate([b_qkv[E + h0 * D:E + (h0 + 1) * D],
                                         b_qkv[E + h1 * D:E + (h1 + 1) * D]]))
        # interleave Q-pair / K-pair chunks: m even = Q, m odd = K
        wqk_l = np.empty((E, 2 * NH * D), dtype=np.float32)
        for p in range(PAIRS):
            wqk_l[:, (2 * p) * P:(2 * p + 1) * P] = qcols[p]
            wqk_l[:, (2 * p + 1) * P:(2 * p + 2) * P] = kcols[p]
        bqk_full = np.empty(2 * NH * D, dtype=np.float32)
        for m in range(2 * PAIRS):
            bqk_full[m * P:(m + 1) * P] = bqk_l[m]
        wv_l = np.concatenate(
            [W_qkv[:, 2 * E + h * D:2 * E + (h + 1) * D] for h in heads], axis=1)
        bv_l = np.concatenate(
            [b_qkv[2 * E + h * D:2 * E + (h + 1) * D] for h in heads])
        wout_l = np.concatenate([W_out[h * D:(h + 1) * D, :] for h in heads],
                                axis=0)
        # pre-tile into the exact SBUF layouts for large-descriptor DMAs
        # (wqk m-chunk-major: [sbuf_partition, m, c, col])
        wqk_t = np.ascontiguousarray(
            wqk_l.reshape(NCC, P, 2 * PAIRS, P).transpose(1, 2, 0, 3)).astype(bf)
        wv_t = np.ascontiguousarray(
            wv_l.reshape(NCC, P, NH * D).transpose(1, 0, 2)).astype(bf)
        wout_t = np.ascontiguousarray(
            wout_l.reshape(PAIRS, P, E).transpose(1, 0, 2)).astype(bf)
        bqk_t = np.ascontiguousarray(
            bqk_full.reshape(NCC, P).T).astype(np.float32)
        per_group.append(dict(
            wqk=wqk_t, wv=wv_t, wout=wout_t,
            bqk=bqk_t, bv=bv_l.astype(bf),
            em=_build_em(heads)))

    in_maps = []
    xt_cache = {}
    for c in range(N_CORES):
        b, g = c // 2, c % 2
        m = dict(per_group[g])
        if b not in xt_cache:
            xt = x[b].T  # [E, S]
            xt_cache[b] = np.ascontiguousarray(
                xt.reshape(NCC, P, NST, ST).transpose(2, 1, 0, 3)).astype(bf)
        m['xT'] = xt_cache[b]
        in_maps.append(m)
    return in_maps


def _run(inputs, trace=False, tmpdir=None, trace_cores=None):
    x = np.asarray(inputs['x'], dtype=np.float32)
    W_qkv = np.asarray(inputs['W_qkv'], dtype=np.float32)
    b_qkv = np.asarray(inputs['b_qkv'], dtype=np.float32)
    W_out = np.asarray(inputs['W_out'], dtype=np.float32)
    b_out = np.asarray(inputs['b_out'], dtype=np.float32)

    nc = _build_program()
    in_maps = _prep_core_inputs(x, W_qkv, b_qkv, W_out, b_out)
    res = bass_utils.run_bass_kernel_spmd(
        nc, in_maps, core_ids=list(range(N_CORES)), trace=trace, tmpdir=tmpdir,
        trace_cores=trace_cores)
    out = np.empty((B, S, E), dtype=np.float32)
    for b in range(B):
        out[b] = res.results[2 * b]['out'] + res.results[2 * b + 1]['out'] + b_out
    return out, res


def kernel(**inputs) -> np.ndarray:
    out, _ = _run(inputs)
    return out


# revision 25
# speedup vs baseline: 1.1109x; 1.1109x over previous
"""Trainium2 Bass kernel: causal self-attention with ALiBi.

Problem: B=4, S=2048, E=1024, H=16, D=64 (fp32 in/out).

Sharding (8 cores): core c -> (batch b = c//2, head-group g = c%2), 8 heads
per group. Each core computes QKV projection for its batch restricted to its
heads, attention for those (b, h), and a partial output projection over its
heads' dims. Host sums the two partials per batch and adds b_out.

On-chip layout (all matmuls bf16 inputs, fp32 PSUM accumulate):
 - QKV projections produce Q and K as PAIR tiles [128, PAIRS, S]: a head
   pair's 2x64 dims live in the 128 partitions (Q pre-scaled by 1/sqrt(D)).
 - Scores for the pair are two ROW-TILED matmuls (K=64 contraction each):
   head0 uses array rows 0-63 (tile_position (0,0)), head1 rows 64-127
   ((64,0)) -- they execute concurrently in the PE array, so a chunk's two
   score matmuls cost ~one matmul of streaming time.
 - P = exp(scores) * EM on scalar/vector; EM is a host-precomputed
   [128, NH, EMW] table carrying exp(slope*(k-q)) * causal_mask.
 - V stored [128, 16, 8, 65] with a ones column so the AV matmul also
   accumulates the softmax denominator l = sum_k P as output row 64.
 - Normalization: l rows for both heads gathered into one [2, ST] tile,
   one reciprocal_approx_fast, broadcast across partitions with a single
   SBUF->SBUF DMA (0-stride source), then attn = num * (1/l) on DVE
   reading the AV psum directly.
 - Output projection psum -> SBUF copies on DVE (scalar engine is the
   attention-phase bottleneck: exp).

Baseline measured 293us; this version targets the scalar bottleneck
(exp-only) and PE score streaming (row tiling).
"""

import sys

if '/opt/trn_rl_repo' not in sys.path:
    sys.path.insert(0, '/opt/trn_rl_repo')

import numpy as np
import ml_dtypes

import concourse.bass as bass
import concourse.tile as tile
from concourse import bacc, mybir
from concourse import bass_utils

BF16 = mybir.dt.bfloat16
F32 = mybir.dt.float32
AF = mybir.ActivationFunctionType

B, S, E, H, D = 4, 2048, 1024, 16, 64
NH = 8          # heads per core
N_CORES = 8
PAIRS = NH // 2
P = 128
ST = 512        # s/q tile (free dim)
NST = S // ST   # 4
NSC = S // P    # 16 s-chunks
NCC = E // P    # 8 contraction chunks
EMW = 2048      # EM table width (q-clipped tiles)
EMOFF = 0

# ALiBi window: keys further than MARGIN/slope below the diagonal contribute
# exp(alibi) < e^-MARGIN of the max element and are skipped at chunk
# granularity.  Head h's slope is 2^-((h+1)/2).  Heads are assigned to the two
# core groups by descending window so both groups' slot s have similar cost;
# the per-slot chunk range is the UNION of the two groups' windows (the
# program is shared by all cores; EM zeros make padded chunks no-ops).
MARGIN = 12.0

SLOT_HEADS_A = [15, 13, 11, 9, 7, 5, 3, 1]
SLOT_HEADS_B = [14, 12, 10, 8, 6, 4, 2, 0]


def _jlo(h, i):
    import math as _m
    delta = MARGIN / (2.0 ** (-(h + 1) / 2.0))
    return max(0, _m.ceil((ST * i - delta - (P - 1)) / P))


def _pair_delta(pr):
    import math as _m
    return max(MARGIN / (2.0 ** (-(h + 1) / 2.0))
               for s in (2 * pr, 2 * pr + 1)
               for h in (SLOT_HEADS_A[s], SLOT_HEADS_B[s]))


def k_chunks_for_pair(pr, i):
    lo = min(_jlo(h, i) for s in (2 * pr, 2 * pr + 1)
             for h in (SLOT_HEADS_A[s], SLOT_HEADS_B[s]))
    return list(range(lo, 4 * i + 4))


def _slopes():
    x = (2.0 ** 8) ** (1.0 / H)
    return np.array([1.0 / x ** (i + 1) for i in range(H)], dtype=np.float64)


def _build_em(heads):
    """EM[p, s, c] = exp(slope_h * m) * [m <= 0], m = k - c  (h = heads[s])."""
    slopes = _slopes()
    k = np.arange(P, dtype=np.float64)[:, None]
    c = np.arange(EMW, dtype=np.float64)[None, :]
    m = k - c
    em = np.zeros((P, NH, EMW), dtype=np.float32)
    for s, h in enumerate(heads):
        v = np.exp(slopes[h] * m) * (m <= 0)
        em[:, s, :] = v.astype(np.float32)
    return em.astype(ml_dtypes.bfloat16)


_PROG_CACHE = {}


def _build_program():
    if 'prog' in _PROG_CACHE:
        return _PROG_CACHE['prog']

    nc = bacc.Bacc("TRN2", target_bir_lowering=False, debug=False,
                   num_devices=N_CORES)

    xT = nc.dram_tensor("xT", [NST, P, NCC, ST], BF16, kind="ExternalInput")
    wqk = nc.dram_tensor("wqk", [P, 2 * PAIRS, NCC, P], BF16, kind="ExternalInput")
    wv = nc.dram_tensor("wv", [P, NCC, NH * D], BF16, kind="ExternalInput")
    wout = nc.dram_tensor("wout", [P, PAIRS, E], BF16, kind="ExternalInput")
    bqk = nc.dram_tensor("bqk", [P, NCC], F32, kind="ExternalInput")
    bv = nc.dram_tensor("bv", [NH * D], BF16, kind="ExternalInput")
    em = nc.dram_tensor("em", [P, NH, EMW], BF16, kind="ExternalInput")
    out = nc.dram_tensor("out", [S, E], F32, kind="ExternalOutput")

    import math as _m

    with tile.TileContext(nc) as tc:
        with (
            tc.tile_pool(name="persist", bufs=1) as pp,
            tc.tile_pool(name="xt", bufs=3) as xtp,
            tc.tile_pool(name="pm", bufs=3) as pmp,
            tc.tile_pool(name="misc", bufs=2) as misc,
            tc.tile_pool(name="osb", bufs=2) as osb,
            tc.tile_pool(name="drp", bufs=8, space="DRAM") as drp,
        ):
            # --- weight / table DMAs, ordered so phase-1 can start ASAP ---
            # wqk is m-chunk-major on the host so the first QKV matmul only
            # waits for a 0.5MB DMA; em must land before attention round 0.
            wqk_sb = pp.tile([P, 2 * PAIRS, NCC, P], BF16, tag="wqk")
            bqk_sb = pp.tile([P, NCC], F32, tag="bqk")
            nc.sync.dma_start(wqk_sb[:, 0:1], wqk.ap()[:, 0:1])
            nc.gpsimd.dma_start(wqk_sb[:, 1:2], wqk.ap()[:, 1:2])
            nc.sync.dma_start(bqk_sb, bqk.ap())
            nc.sync.dma_start(wqk_sb[:, 2:4], wqk.ap()[:, 2:4])
            nc.gpsimd.dma_start(wqk_sb[:, 4:6], wqk.ap()[:, 4:6])
            nc.sync.dma_start(wqk_sb[:, 6:8], wqk.ap()[:, 6:8])
            wv_sb = pp.tile([P, NCC, NH * D], BF16, tag="wv")
            nc.gpsimd.dma_start(wv_sb, wv.ap())
            em_sb = pp.tile([P, NH, EMW], BF16, tag="em")
            for _q in range(4):
                _sl = slice(_q * 2, _q * 2 + 2)
                _eng = nc.sync if _q % 2 == 0 else nc.gpsimd
                _eng.dma_start(em_sb[:, _sl], em.ap()[:, _sl])
            wout_sb = pp.tile([P, PAIRS, E], BF16, tag="wout")
            nc.gpsimd.dma_start(wout_sb, wout.ap())

            bv_bc = pp.tile([P, NH * D], BF16, tag="bvbc")
            q_sb = pp.tile([P, PAIRS, S], BF16, tag="q")
            k_sb = pp.tile([P, PAIRS, S], BF16, tag="k")
            v_sb = pp.tile([P, NSC, NH, D + 1], BF16, tag="v")
            attn_sb = pp.tile([P, PAIRS, S], BF16, tag="attn")

            nc.vector.memset(v_sb[:, :, :, D:D + 1], 1.0)

            p1 = tc.tile_pool(name="qkvps", bufs=2, space="PSUM")
            qkps = p1.__enter__()
            p2a = tc.tile_pool(name="scps", bufs=2, space="PSUM")
            scps = p2a.__enter__()
            p2b = tc.tile_pool(name="avps", bufs=1, space="PSUM")
            avps = p2b.__enter__()

            xt_tiles = {}

            def issue_xt_dma(i):
                xt = xtp.tile([P, NCC, ST], BF16, tag="xt")
                nc.scalar.dma_start(xt[:, 0:NCC // 2], xT.ap()[i][:, 0:NCC // 2])
                nc.scalar.dma_start(xt[:, NCC // 2:], xT.ap()[i][:, NCC // 2:])
                xt_tiles[i] = xt
                if i == 0:
                    # tiny, needed by the first V bias add (~30us in); the
                    # scalar queue is otherwise idle here.
                    nc.scalar.dma_start(
                        bv_bc, bv.ap()[None, :].to_broadcast([P, NH * D]))

            def qkv_steps(i):
                """Per-step closures for tile i's QKV projections."""
                xt = xt_tiles.pop(i)
                sl = slice(i * ST, (i + 1) * ST)
                st = {}
                steps = []
                for m in range(2 * PAIRS):
                    for c in range(NCC):
                        def f(m=m, c=c):
                            if c == 0:
                                st[m] = qkps.tile([P, ST], F32, tag="ps", name="ps")
                            nc.tensor.matmul(
                                st[m], lhsT=wqk_sb[:, m, c, :],
                                rhs=xt[:, c, :],
                                start=(c == 0), stop=(c == NCC - 1))
                        steps.append(f)

                    def fa(m=m):
                        pr = m // 2
                        dst = q_sb if m % 2 == 0 else k_sb
                        nc.scalar.activation(dst[:, pr, sl], st.pop(m),
                                             AF.Identity, bias=bqk_sb[:, m:m + 1])
                    steps.append(fa)
                for s4 in range(4):
                    sc = 4 * i + s4
                    for c in range(NCC):
                        def f(s4=s4, c=c):
                            if c == 0:
                                st['v'] = qkps.tile([P, NH * D], F32, tag="ps", name="ps")
                            nc.tensor.matmul(
                                st['v'], lhsT=xt[:, c, s4 * P:(s4 + 1) * P],
                                rhs=wv_sb[:, c, :],
                                start=(c == 0), stop=(c == NCC - 1))
                        steps.append(f)

                    def fv(sc=sc):
                        vp = st.pop('v')
                        nc.vector.tensor_add(
                            out=v_sb[:, sc, :, 0:D],
                            in0=vp.rearrange("p (h d) -> p h d", h=NH),
                            in1=bv_bc.rearrange("p (h d) -> p h d", h=NH))
                    steps.append(fv)
                return steps

            def outproj_steps(i):
                """Per-step closures for tile i's output projection."""
                st = {}
                steps = []
                for qc in range(4 * i, 4 * i + 4):
                    for n in range(E // ST):
                        for p_ in range(PAIRS):
                            def f(qc=qc, n=n, p_=p_):
                                if p_ == 0:
                                    st['op'] = qkps.tile([P, ST], F32, tag="ps", name="ps")
                                nc.tensor.matmul(
                                    st['op'],
                                    lhsT=attn_sb[:, p_, qc * P:(qc + 1) * P],
                                    rhs=wout_sb[:, p_, n * ST:(n + 1) * ST],
                                    start=(p_ == 0), stop=(p_ == PAIRS - 1))
                            steps.append(f)

                        def fo(qc=qc, n=n):
                            op = st.pop('op')
                            ot = osb.tile([P, ST], F32, tag="ot")
                            nc.vector.tensor_copy(ot, op)
                            nc.sync.dma_start(
                                out.ap()[qc * P:(qc + 1) * P,
                                         n * ST:(n + 1) * ST], ot)
                        steps.append(fo)
                return steps

            def attn_round(i, filler):
                """Attention for tile i, draining `filler` steps between
                matmuls to keep the PE busy through the exp/EM dep chain."""
                qsl = slice(i * ST, (i + 1) * ST)
                nslots = 2 * sum(len(k_chunks_for_pair(pr, i))
                                 for pr in range(PAIRS)) + 2 * PAIRS
                slots_left = [nslots]
                pending = []

                def fill():
                    if pending:
                        pending.pop(0)()
                    k = -(-len(filler) // slots_left[0]) if filler else 0
                    for _ in range(k):
                        filler.pop(0)()
                    slots_left[0] -= 1

                for pr in range(PAIRS):
                    s0 = 2 * pr
                    delta = int(_m.ceil(_pair_delta(pr)))
                    av0 = avps.tile([P, ST], F32, tag="av0")
                    av1 = avps.tile([P, ST], F32, tag="av1")
                    js = k_chunks_for_pair(pr, i)
                    for jj, j in enumerate(js):
                        # causal clip below the diagonal; ALiBi-window clip
                        # above it (first chunk stays full-width so the AV
                        # accumulation start covers every column).
                        qlo = max(0, P * j - ST * i)
                        qhi = ST if jj == 0 else min(
                            ST, P * j + P + delta - ST * i)
                        w = qhi - qlo
                        qs = slice(i * ST + qlo, i * ST + qhi)
                        jsl = slice(j * P, (j + 1) * P)
                        scp = scps.tile([P, 2 * ST], F32, tag="scp")
                        scp3 = scp.rearrange("p (t st) -> p t st", t=2)
                        # row-tiled pair: head0 in array rows 0-63, head1 in
                        # rows 64-127 -- concurrent in the PE array.
                        nc.tensor.matmul(
                            scp[:, 0:w], lhsT=k_sb[0:D, pr, jsl],
                            rhs=q_sb[0:D, pr, qs], start=True, stop=True)
                        nc.tensor.matmul(
                            scp[:, ST:ST + w], lhsT=k_sb[D:P, pr, jsl],
                            rhs=q_sb[D:P, pr, qs], start=True, stop=True)
                        fill()
                        pm = pmp.tile([P, 2 * ST], BF16, tag="pm")
                        pm3 = pm.rearrange("p (t st) -> p t st", t=2)
                        nc.scalar.activation(pm3[:, :, 0:w], scp3[:, :, 0:w],
                                             AF.Exp)
                        pm2 = pmp.tile([P, 2 * ST], BF16, tag="pm2")
                        pm23 = pm2.rearrange("p (t st) -> p t st", t=2)
                        a0 = EMOFF - P * j + ST * i + qlo
                        nc.vector.tensor_mul(
                            pm23[:, :, 0:w], pm3[:, :, 0:w],
                            em_sb[:, s0:s0 + 2, a0:a0 + w])
                        nc.tensor.matmul(
                            av0[0:D + 1, qlo:qhi], lhsT=v_sb[:, j, s0, :],
                            rhs=pm2[:, 0:w],
                            start=(jj == 0), stop=(jj == len(js) - 1))
                        nc.tensor.matmul(
                            av1[0:D + 1, qlo:qhi], lhsT=v_sb[:, j, s0 + 1, :],
                            rhs=pm2[:, ST:ST + w],
                            start=(jj == 0), stop=(jj == len(js) - 1))
                        fill()
                    # evacuate AV psum (incl. the l row) promptly on the
                    # scalar engine so the next pair can reuse the banks;
                    # the rest of the normalization (1/l broadcast via DRAM
                    # round trip, final mul) is deferred into the next
                    # pair's chunk slots to keep the vector queue flowing.
                    for odd, av in ((0, av0), (1, av1)):
                        avs = misc.tile([D + 1, ST], F32, tag=f"avs{odd}",
                                        name="avs")
                        nc.scalar.activation(avs, av[0:D + 1, :], AF.Copy)
                        stn = {}

                        def s1(odd=odd, avs=avs, stn=stn):
                            scratch = drp.tile([1, ST], F32, tag=f"lscr{odd}",
                                               name="lscr")
                            nc.sync.dma_start(scratch, avs[D:D + 1, :])
                            lb = misc.tile([D, ST], F32, tag=f"lbc{odd}",
                                           name="lbc")
                            nc.sync.dma_start(
                                lb,
                                bass.AP(tensor=scratch.tensor,
                                        offset=scratch.offset,
                                        ap=[[0, D]] + scratch.ap[1:]))
                            stn['lb'] = lb

                        def s2(stn=stn):
                            nc.vector.reciprocal_approx_fast(out=stn['lb'],
                                                             in_=stn['lb'])

                        def s3(odd=odd, avs=avs, stn=stn, pr=pr):
                            lb = stn.pop('lb')
                            if not odd:
                                nc.vector.tensor_mul(attn_sb[0:D, pr, qsl],
                                                     avs[0:D, :], lb)
                            else:
                                atmp = misc.tile([D, ST], BF16, tag="atmp",
                                                 name="atmp")
                                nc.vector.tensor_mul(atmp, avs[0:D, :], lb)
                                nc.sync.dma_start(attn_sb[D:P, pr, qsl], atmp)

                        pending.extend([s1, s2, s3])
                        fill()
                while pending:
                    pending.pop(0)()
                while filler:
                    filler.pop(0)()

            # software-pipelined schedule:
            #   r0: qkv(0);  r1: attn(0)+qkv(1);  r2: attn(1)+qkv(2)+outproj(0)
            #   r3: attn(2)+qkv(3)+outproj(1);  r4: attn(3)+outproj(2);
            #   tail: outproj(3)
            issue_xt_dma(0)
            issue_xt_dma(1)
            for f in qkv_steps(0):
                f()
            issue_xt_dma(2)
            attn_round(0, qkv_steps(1))
            issue_xt_dma(3)
            attn_round(1, qkv_steps(2) + outproj_steps(0))
            attn_round(2, qkv_steps(3))
            attn_round(3, outproj_steps(1) + outproj_steps(2))
            for f in outproj_steps(3):
                f()
            p2b.__exit__(None, None, None)
            p2a.__exit__(None, None, None)
            p1.__exit__(None, None, None)

    nc.compile()
    _PROG_CACHE['prog'] = nc
    return nc


def _head_groups():
    return [SLOT_HEADS_A, SLOT_HEADS_B]


def _prep_core_inputs(x, W_qkv, b_qkv, W_out, b_out):
    """Build the 8 per-core input dicts (host-side shard + transform)."""
    groups = _head_groups()
    bf = ml_dtypes.bfloat16
    per_group = []
    for heads in groups:
        qcols = []
        kcols = []
        bqk_l = []
        for p in range(PAIRS):
            h0, h1 = heads[2 * p], heads[2 * p + 1]
            wq = np.concatenate([W_qkv[:, h0 * D:(h0 + 1) * D],
                                 W_qkv[:, h1 * D:(h1 + 1) * D]], axis=1) / 8.0
            wk = np.concatenate([W_qkv[:, E + h0 * D:E + (h0 + 1) * D],
                                 W_qkv[:, E + h1 * D:E + (h1 + 1) * D]], axis=1)
            qcols.append(wq)
            kcols.append(wk)
            bqk_l.append(np.concatenate([b_qkv[h0 * D:(h0 + 1) * D],
                                         b_qkv[h1 * D:(h1 + 1) * D]]) / 8.0)
            bqk_l.append(np.concatenate([b_qkv[E + h0 * D:E + (h0 + 1) * D],
                                         b_qkv[E + h1 * D:E + (h1 + 1) * D]]))
        # interleave Q-pair / K-pair chunks: m even = Q, m odd = K
        wqk_l = np.empty((E, 2 * NH * D), dtype=np.float32)
        for p in range(PAIRS):
            wqk_l[:, (2 * p) * P:(2 * p + 1) * P] = qcols[p]
            wqk_l[:, (2 * p + 1) * P:(2 * p + 2) * P] = kcols[p]
        bqk_full = np.empty(2 * NH * D, dtype=np.float32)
        for m in range(2 * PAIRS):
            bqk_full[m * P:(m + 1) * P] = bqk_l[m]
        wv_l = np.concatenate(
            [W_qkv[:, 2 * E + h * D:2 * E + (h + 1) * D] for h in heads], axis=1)
        bv_l = np.concatenate(
            [b_qkv[2 * E + h * D:2 * E + (h + 1) * D] for h in heads])
        wout_l = np.concatenate([W_out[h * D:(h + 1) * D, :] for h in heads],
                                axis=0)
        # pre-tile into the exact SBUF layouts for large-descriptor DMAs
        # (wqk m-chunk-major: [sbuf_partition, m, c, col])
        wqk_t = np.ascontiguousarray(
            wqk_l.reshape(NCC, P, 2 * PAIRS, P).transpose(1, 2, 0, 3)).astype(bf)
        wv_t = np.ascontiguousarray(
            wv_l.reshape(NCC, P, NH * D).transpose(1, 0, 2)).astype(bf)
        wout_t = np.ascontiguousarray(
            wout_l.reshape(PAIRS, P, E).transpose(1, 0, 2)).astype(bf)
        bqk_t = np.ascontiguousarray(
            bqk_full.reshape(NCC, P).T).astype(np.float32)
        per_group.append(dict(
            wqk=wqk_t, wv=wv_t, wout=wout_t,
            bqk=bqk_t, bv=bv_l.astype(bf),
            em=_build_em(heads)))

    in_maps = []
    xt_cache = {}
    for c in range(N_CORES):
        b, g = c // 2, c % 2
        m = dict(per_group[g])
        if b not in xt_cache:
            xt = x[b].T  # [E, S]
            xt_cache[b] = np.ascontiguousarray(
                xt.reshape(NCC, P, NST, ST).transpose(2, 1, 0, 3)).astype(bf)
        m['xT'] = xt_cache[b]
        in_maps.append(m)
    return in_maps


def _run(inputs, trace=False, tmpdir=None, trace_cores=None):
    x = np.asarray(inputs['x'], dtype=np.float32)
    W_qkv = np.asarray(inputs['W_qkv'], dtype=np.float32)
    b_qkv = np.asarray(inputs['b_qkv'], dtype=np.float32)
    W_out = np.asarray(inputs['W_out'], dtype=np.float32)
    b_out = np.asarray(inputs['b_out'], dtype=np.float32)

    nc = _build_program()
    in_maps = _prep_core_inputs(x, W_qkv, b_qkv, W_out, b_out)
    res = bass_utils.run_bass_kernel_spmd(
        nc, in_maps, core_ids=list(range(N_CORES)), trace=trace, tmpdir=tmpdir,
        trace_cores=trace_cores)
    out = np.empty((B, S, E), dtype=np.float32)
    for b in range(B):
        out[b] = res.results[2 * b]['out'] + res.results[2 * b + 1]['out'] + b_out
    return out, res


def kernel(**inputs) -> np.ndarray:
    out, _ = _run(inputs)
    return out
